# revision 38
# baseline (speedup 1.0000x reference)
"""Trainium2 Bass kernel for a single-head attention layer (folded form).

Problem: x [4, 2048, 1024] f32; torch-Linear qkv (W_qkv [3072, 1024]) ->
single-head attention (d=1024) -> output projection (W_proj [1024, 1024]).

Algebraic folds (exact, host-side fp32):
  scores*scale = q.k*scale = x (scale Wq^T Wk) x^T + beta_j + const(query)
    with beta = x @ (scale Wk^T b_q)  (per-key activation bias; the
    query-constant term and the k-bias term cancel in softmax)
  y = (A V) Wp^T = A (x (Wp Wv)^T)  -- proj folded into the V weights.
So the device only computes:
  Q' = x M          (M = scale Wq^T Wk, 128 MMs)
  V'' = x Wvp^T     (Wvp = Wp Wv, 128 MMs)
  S^T = x^T-blocks . Q'^T -> exp(.+beta)   (256 MMs)
  y^T = V''^T-blocks . exp  (256 MMs)
768 N=512 matmuls/core vs 1024 for the direct form: the K^T projection and
the output projection phases are gone entirely.

Sharding: 8 NeuronCores = 4 batches x 2 token-halves. Keys/values for the
partner half arrive via two pairwise AllGathers (replica groups
[[0,1],..]): one 2MB gather of the raw x^T (B's partner stationary data,
mirrored SBUF->DRAM right after the input load since collectives cannot
read IO tensors), and one 2MB gather of V''. The pairwise CC stream is
serial (start barrier, then ops back-to-back), so few large ops keep the
worst-case chain short when peer launch skew delays the barrier.
Exchanged data is consumed as late as possible (B_own -> C_own ->
B_partner -> C_partner) so peer launch skew + collective jitter stays
hidden under compute.

Performance structure (steady state ~216ns per N=512 bf16 matmul):
  - Q' and V'' run as two d-major 8-bank sweeps each, consuming their
    weight chunks in DMA arrival order; 46 dummy warmup matmuls bridge
    the PE to the input-DMA gate (first (m0,x) chunk pair ~14us). The m
    matrix is split into per-sweep column halves (m0/m1) so sweep 0 only
    gates on 3MB of stream.
  - Phase order Q' -> V'' -> B_own -> C_own -> B_par -> C_par: V''
    right after Q' maximizes peer-skew tolerance (the partner's V''
    gather can lag ~80us before C_par feels it), while B_own (which
    needs no input bytes at all) buffers C_own from any stream jitter.
  - All input DMAs are issued on the Sync queue in consumption order
    (m0/x interleaved with m1's first chunk hoisted, then beta, m1, wvp).
    One queue sustains ~300GB/s early / ~420GB/s late; a second queue
    de-pairs arrivals (tried, slower), and 512KB descriptors stream
    measurably faster than 256KB halves (200-260GB/s), so x chunks stay
    whole even though that delays the first matmul slightly.
  - PSUM pools span the whole kernel; accumulation-bank pairs rotate
    [s, av01, s, av23] so a bank is reused only every 4th group.
  - The final C pass is bank-major and its last bank drains in two
    256-col pieces on two DMA queues, so the exposed tail is one
    quarter-add plus one small DMA's completion latency.
  - PSUM->SBUF drains split across ScalarE and VectorE; y^T leaves as
    bf16 (host normalizes in f32).

Measured: 184.3-185.9us typical, rare peer-skew outliers to ~195-220us
when another core's NEFF launches ~40-60us late (the collective start
barrier absorbs the skew; consumption order caps the damage). (vs
242.6us for the direct-form baseline; bf16 matmul
roofline for the folded form is ~164us/core, plus ~7.7us framework
preamble, ~5.7us input-DMA gate, ~3.2us hardware duty-cycle throttle
(432ns every ~10.7us, drifts across program structure - timer-based),
~2.5us ldweights handoff overhead (216.6 vs 213.3ns/MM; walrus
ldw-opt is incompatible with framework-emitted InstLdweights), and
~3us output-DMA tail.
"""

import math

import numpy as np
import ml_dtypes

import concourse.bass as bass
import concourse.tile as tile
from concourse import mybir
from concourse.bass_utils import run_bass_kernel_spmd
from concourse.vector_clock import ScopedClock, VectorClock

BF16 = mybir.dt.bfloat16
F32 = mybir.dt.float32
AF = mybir.ActivationFunctionType

D = 1024   # model dim
S = 2048   # sequence length
Q = 1024   # queries per core
H = 1024   # keys per core (own half)
P = 128    # SBUF partitions
NB = 512   # matmul moving-block size
DT = D // P
HT = H // P
ST = S // P
N_CORES = 8
GROUPS = [[0, 1], [2, 3], [4, 5], [6, 7]]
WARMUP_MMS = 46  # bridges the PE to the input-DMA gate (first d-tile pair)

# ---------------------------------------------------------------------------
# Workarounds for this container's walrus, which rejects any instruction
# carrying more than one sem wait ("Too many sync wait commands").
# ---------------------------------------------------------------------------


def _patched_drain_and_barrier(self, tick_clock, wait_clock):
    # Split the kernel-tail drain into one drain per semaphore (1 wait each).
    gc = tick_clock.global_clock
    n = len(gc)
    for i in range(n):
        if gc[i] > 0:
            vec = [0] * n
            vec[i] = gc[i]
            dr = self.nc.sync.drain()
            wait_clock.add_sem_waits(dr.ins, ScopedClock({None: VectorClock(vec)}))
    # sem-only barriers: the per-sem drains above already imply all
    # sem-updating work has completed; skip the per-engine InstDrains.
    self.nc.all_engine_barrier(sem_only=True)
    popped = self.nc._tile_sem_poison_stack.pop()
    assert popped is self._sem_poison
    self.nc.clear_and_free_semaphores(list(self.sems.allocated().values()))
    self.nc.all_engine_barrier(sem_only=True)


_MAX_WAITS = 1
_split_counter = [0]


def _split_excess_waits(ordered):
    # Hoist excess waits onto preceding same-engine NoOps.
    for insts in ordered.values():
        new_list = []
        for inst in insts:
            si = inst.sync_info
            waits = list(si.on_wait) if si is not None and si.on_wait else []
            if len(waits) > _MAX_WAITS and inst.engine is not None:
                extra, keep = waits[:-_MAX_WAITS], waits[-_MAX_WAITS:]
                for w in extra:
                    _split_counter[0] += 1
                    nop = mybir.InstNoOp(
                        name=f"waitsplit-{_split_counter[0]}",
                        sync_info=mybir.SyncInfo(on_wait=[w], on_update=[]),
                        bass_nofuse=True,
                        engine=inst.engine,
                    )
                    new_list.append(nop)
                inst.sync_info = mybir.SyncInfo(
                    on_wait=keep, on_update=list(si.on_update))
            new_list.append(inst)
        insts[:] = new_list


def _install_patches():
    if getattr(tile.TileContext, "_attn_patched", False):
        return
    tile.TileContext._drain_and_barrier = _patched_drain_and_barrier
    orig_lower = tile.TileContext._lower_ordered_insts

    def _lower_with_wait_split(self, ordered):
        _split_excess_waits(ordered)
        return orig_lower(self, ordered)

    tile.TileContext._lower_ordered_insts = _lower_with_wait_split
    tile.TileContext._attn_patched = True


_install_patches()

# ---------------------------------------------------------------------------
# Device program
# ---------------------------------------------------------------------------


def build_nc():
    nc = bass.Bass("TRN2", target_bir_lowering=False, debug=False,
                   num_devices=N_CORES)

    # All weight-like tensors are packed partition-major in 2-d-tile chunks
    # (rows r = 128*b + p hold d-tiles (2b, 2b+1) side by side): 4KB DMA
    # lines, and d-major sweeps consume chunks in arrival order.
    xt = nc.dram_tensor("xt", [D // 2, 2 * H], BF16, kind="ExternalInput").ap()
    m0 = nc.dram_tensor("m0", [D // 2, D], BF16, kind="ExternalInput").ap()
    m1 = nc.dram_tensor("m1", [D // 2, D], BF16, kind="ExternalInput").ap()
    wvp = nc.dram_tensor("wvp", [D // 2, 2 * D], BF16,
                         kind="ExternalInput").ap()
    beta = nc.dram_tensor("beta", [P, ST], F32, kind="ExternalInput").ap()
    yt = nc.dram_tensor("yt", [D, Q], BF16, kind="ExternalOutput").ap()
    sums = nc.dram_tensor("sums", [1, Q], F32, kind="ExternalOutput").ap()

    x_mir = nc.dram_tensor("x_mir", [D // 2, 2 * H], BF16).ap()
    x_recv = nc.dram_tensor("x_recv", [D, 2 * H], BF16).ap()
    v_send = nc.dram_tensor("v_send", [H, D], BF16).ap()
    v_recv = nc.dram_tensor("v_recv", [2 * H, D], BF16).ap()

    from contextlib import ExitStack
    with tile.TileContext(nc) as tc, ExitStack() as stack:
        res = stack.enter_context(tc.tile_pool(name="res", bufs=1))
        # x^T own chunks persist into phase B (B_own stationary data).
        xch = [res.tile([P, 2 * H], BF16, tag=f"x{b}", name=f"x{b}")
               for b in range(DT // 2)]
        qt_t = [res.tile([P, Q], BF16, tag=f"qt{e}", name=f"qt{e}")
                for e in range(DT)]
        v_own = [res.tile([P, D], BF16, tag=f"vo{j}", name=f"vo{j}")
                 for j in range(HT)]
        beta_t = res.tile([P, ST], F32, tag="beta", name="beta")

        # PSUM pools span the whole kernel: closing a PSUM pool inserts a
        # full PE-drain barrier before the next phase can reuse the banks.
        scr_ps = stack.enter_context(
            tc.tile_pool(name="scr_ps", bufs=2, space="PSUM"))
        av_ps = stack.enter_context(
            tc.tile_pool(name="av_ps", bufs=1, space="PSUM"))
        pair_cycle = [0]

        def next_pair(nm):
            # Rotate accumulation-bank pairs [s, av01, s, av23] so a bank is
            # reused only every 4th group - its PSUM->SBUF drain is long done.
            mmod = pair_cycle[0] % 4
            pair_cycle[0] += 1
            if mmod == 1:
                tags = ("av0", "av1")
                pool = av_ps
            elif mmod == 3:
                tags = ("av2", "av3")
                pool = av_ps
            else:
                tags = ("s0", "s1")
                pool = scr_ps
            return [pool.tile([P, NB], F32, tag=t, name=f"{nm}_{i}")
                    for i, t in enumerate(tags)]

        def sweep8(nm):
            return ([scr_ps.tile([P, NB], F32, tag=t, name=f"{nm}_{i}")
                     for i, t in enumerate(("s0", "s1", "s0", "s1"))]
                    + [av_ps.tile([P, NB], F32, tag=f"av{i}",
                                  name=f"{nm}_av{i}") for i in range(4)])

        # ---------------- Phase A: projections + exchanges ----------------
        if True:
            mch = [[res.tile([P, D], BF16, tag=f"m{s}{b}", name=f"m{s}{b}")
                    for b in range(DT // 2)] for s in range(2)]
            wch = [res.tile([P, 2 * D], BF16, tag=f"wvp{b}", name=f"wvp{b}")
                   for b in range(DT // 2)]

            def xt_sl(d, sl):
                base = (d % 2) * H
                return xch[d // 2][:, base + sl.start:base + sl.stop]

            def m_sl(s, d, sl):
                # sweep-half s, d-tile d, e'-columns sl within the half
                base = (d % 2) * (D // 2)
                return mch[s][d // 2][:, base + sl.start:base + sl.stop]

            def wvp_sl(d, sl):
                base = (d % 2) * D
                return wch[d // 2][:, base + sl.start:base + sl.stop]

            warm = res.tile([P, P], BF16, tag="warm", name="warm")
            nc.vector.memset(warm, 0.125)

            # All input DMAs on the Sync queue in consumption order: the
            # queue transfers FIFO (two queues were tried and are slower -
            # the early DMA path is a shared bottleneck and interleaving
            # de-pairs the m/x arrivals); wvp streams after the m/x chunks,
            # well before V''. The first chunks are split at d-tile
            # granularity so Q' can start half a chunk earlier.
            for b in range(DT // 2):
                nc.sync.dma_start(out=mch[0][b], in_=m0[b * P:(b + 1) * P, :])
                if b == DT // 2 - 1:
                    # sweep 1's first chunk jumps ahead of the last x chunk:
                    # it is needed ~3.5us after d=6's data, but in FIFO order
                    # it would land a whole chunk later.
                    nc.sync.dma_start(out=mch[1][0], in_=m1[0:P, :])
                # whole 512KB x chunks: large descriptors sustain a visibly
                # higher early stream rate than 256KB halves
                nc.sync.dma_start(out=xch[b], in_=xt[b * P:(b + 1) * P, :])
            nc.sync.dma_start(out=beta_t, in_=beta[:, :])
            for b in range(1, DT // 2):
                nc.sync.dma_start(out=mch[1][b], in_=m1[b * P:(b + 1) * P, :])
            for b in range(DT // 2):
                nc.sync.dma_start(out=wch[b], in_=wvp[b * P:(b + 1) * P, :])

            # Dummy matmuls on the memset tile: keep the PE busy while the
            # first m/x chunks stream in so HAM reaches K=8/8.
            wm_ps = scr_ps.tile([P, NB], F32, tag="s0", name="wm")
            for i in range(WARMUP_MMS):
                nc.tensor.matmul(wm_ps[:, 0:P], warm, warm,
                                 start=(i == 0), stop=(i == WARMUP_MMS - 1))

            # Q'^T[e, q] in two d-major 8-bank sweeps of 4 e-tiles x 2
            # halves, so the PE consumes m[d]/xt[d] in DMA arrival order
            # instead of stalling a whole e-group on the last d-tile.
            for s in range(2):
                ps8 = sweep8(f"qp{s}")
                for d in range(DT):
                    for i in range(4):
                        e = s * 4 + i
                        for h in range(2):
                            nc.tensor.matmul(
                                ps8[i * 2 + h],
                                m_sl(s, d, slice(i * P, (i + 1) * P)),
                                xt_sl(d, slice(h * NB, (h + 1) * NB)),
                                start=(d == 0), stop=(d == DT - 1))
                for i in range(4):
                    e = s * 4 + i
                    nc.scalar.activation(
                        out=qt_t[e][:, 0:NB], in_=ps8[i * 2],
                        func=AF.Identity)
                    nc.vector.tensor_copy(
                        out=qt_t[e][:, NB:2 * NB], in_=ps8[i * 2 + 1])

            # x^T exchange: collectives cannot read IO tensors, so mirror
            # the loaded chunks SBUF->DRAM on the Scalar queue. Issued
            # after the Q' sweeps so the 2MB of mirror writes never touch
            # HBM during the input-stream gate; partner x^T is consumed
            # last (B_partner), leaving ~90us of slack.
            for b in range(DT // 2):
                nc.scalar.dma_start(out=x_mir[b * P:(b + 1) * P, :],
                                    in_=xch[b])
            # One merged 2MB gather: the pairwise CC stream is SERIAL
            # (barrier, then each op back-to-back at ~10-17us apiece), so
            # fewer, larger ops shorten the worst-case chain when peer
            # launch skew delays the stream's start barrier.
            nc.gpsimd.collective_compute(
                "AllGather", mybir.AluOpType.bypass,
                replica_groups=GROUPS,
                ins=[x_mir[:, :]],
                outs=[x_recv[:, :]])

        # ---------------- Phases B, C ----------------
        if True:
            phb = res
            # Partner-half import tiles (filled after V'' below).
            xt_par = phb.tile([P, DT // 2, 2 * H], BF16, tag="xp", name="xp")
            v_par = phb.tile([P, HT, D], BF16, tag="vp", name="vp")

            exp_t = [phb.tile([P, Q], BF16, tag=f"exp{j}", name=f"exp{j}")
                     for j in range(ST)]
            y_acc = [phb.tile([P, Q], BF16, tag=f"ya{e}", name=f"ya{e}")
                     for e in range(DT)]

            # f32 per-key-lane partial sums, accumulated on the (idle) DVE;
            # cast to bf16 and reduced across partitions with two cheap
            # bf16 matmuls.
            sumacc = phb.tile([P, Q], F32, tag="sumacc", name="sumacc")
            sumacc_bf = phb.tile([P, Q], BF16, tag="sumbf", name="sumbf")
            ones_b = phb.tile([P, 1], BF16, tag="ones_b", name="ones_b")
            nc.vector.memset(ones_b, 1.0)

            def xkey_sl(e, j):
                # stationary [128 d-rows of e-block, 128 key-tokens]
                if j < HT:
                    return xt_sl(e, slice(j * P, (j + 1) * P))
                jj = j - HT
                base = (e % 2) * H
                return xt_par[:, e // 2, base + jj * P:base + (jj + 1) * P]

            def v_tile(j):
                if j < HT:
                    return v_own[j]
                return v_par[:, j - HT, :]

            # Exchanged data is consumed as late as possible:
            #   B_own -> C_own (own keys only) -> B_partner -> C_partner.
            def b_group(j):
                ps = next_pair(f"bps{j}")
                for e in range(DT):
                    for qb in range(2):
                        nc.tensor.matmul(
                            ps[qb], xkey_sl(e, j),
                            qt_t[e][:, qb * NB:(qb + 1) * NB],
                            start=(e == 0), stop=(e == DT - 1))
                for qb in range(2):
                    sl = slice(qb * NB, (qb + 1) * NB)
                    nc.scalar.activation(
                        out=exp_t[j][:, sl], in_=ps[qb], func=AF.Exp,
                        bias=beta_t[:, j:j + 1])
                    if j == 0:
                        nc.vector.tensor_copy(
                            out=sumacc[:, sl], in_=exp_t[j][:, sl])
                    else:
                        nc.vector.tensor_add(
                            sumacc[:, sl], sumacc[:, sl], exp_t[j][:, sl])

            def c_pass(p, half, last=False):
                # 2 e-tiles x 2 q-halves = 4 PSUM banks, s/av sets
                # alternating so a pass never waits on the previous drain.
                # half 0 writes y_acc; half 1 accumulates into it (DVE) and
                # the finished y^T tiles stream out.
                if p % 2 == 0:
                    ps_o = [scr_ps.tile([P, NB], F32, tag=f"s{i % 2}",
                                        name=f"cps{half}_{p}_{i}")
                            for i in range(4)]
                else:
                    ps_o = [av_ps.tile([P, NB], F32, tag=f"av{i}",
                                       name=f"cps{half}_{p}_{i}")
                            for i in range(4)]
                j0 = half * HT

                def mms(i, qb, j):
                    nc.tensor.matmul(
                        ps_o[i * 2 + qb],
                        v_tile(j)[:, (p * 2 + i) * P:(p * 2 + i + 1) * P],
                        exp_t[j][:, qb * NB:(qb + 1) * NB],
                        start=(j == j0), stop=(j == j0 + HT - 1))

                def drain(i, qb):
                    dp = p * 2 + i
                    sl = slice(qb * NB, (qb + 1) * NB)
                    if half == 0:
                        if qb == 0:
                            nc.scalar.activation(out=y_acc[dp][:, sl],
                                                 in_=ps_o[i * 2],
                                                 func=AF.Identity)
                        else:
                            nc.vector.tensor_copy(out=y_acc[dp][:, sl],
                                                  in_=ps_o[i * 2 + 1])
                    elif last and i == 1 and qb == 1:
                        # Final bank: drain in two 256-col pieces on two DMA
                        # queues so the exposed tail is one quarter-add +
                        # two overlapped small DMAs.
                        for t in range(2):
                            tsl = slice(qb * NB + t * (NB // 2),
                                        qb * NB + (t + 1) * (NB // 2))
                            psl = slice(t * (NB // 2), (t + 1) * (NB // 2))
                            nc.vector.tensor_add(
                                y_acc[dp][:, tsl], y_acc[dp][:, tsl],
                                ps_o[i * 2 + qb][:, psl])
                            eng = nc.scalar if t == 0 else nc.sync
                            eng.dma_start(
                                out=yt[dp * P:(dp + 1) * P, tsl],
                                in_=y_acc[dp][:, tsl])
                    else:
                        nc.vector.tensor_add(
                            y_acc[dp][:, sl], y_acc[dp][:, sl],
                            ps_o[i * 2 + qb])
                        eng = nc.scalar if qb == 0 else nc.sync
                        eng.dma_start(out=yt[dp * P:(dp + 1) * P, sl],
                                      in_=y_acc[dp][:, sl])

                if last:
                    # Bank-major: each bank's 8-key-tile accumulation
                    # finishes 8 matmul slots before the next one, so its
                    # DVE add + y^T DMA overlap the remaining matmuls and
                    # only the final bank's drain is exposed in the tail.
                    for i in range(2):
                        for qb in range(2):
                            for j in range(j0, j0 + HT):
                                mms(i, qb, j)
                            drain(i, qb)
                else:
                    for j in range(j0, j0 + HT):
                        for i in range(2):
                            for qb in range(2):
                                mms(i, qb, j)
                    for i in range(2):
                        for qb in range(2):
                            drain(i, qb)
                return ps_o

            # V''[token, e] -> SBUF (kept) + v_send; AllGather per half.
            # Two d-major 8-bank sweeps (4 token-tiles x 2 e-halves) so the
            # PE consumes wvp[d] in DMA arrival order - sweep 0 starts as
            # soon as wvp chunk 0 lands instead of stalling on the last one.
            for s in range(2):
                ps8 = sweep8(f"vp{s}")
                for d in range(DT):
                    for jj in range(4):
                        j = s * 4 + jj
                        for eb in range(2):
                            nc.tensor.matmul(
                                ps8[jj * 2 + eb],
                                xt_sl(d, slice(j * P, (j + 1) * P)),
                                wvp_sl(d, slice(eb * NB, (eb + 1) * NB)),
                                start=(d == 0), stop=(d == DT - 1))
                for jj in range(4):
                    j = s * 4 + jj
                    nc.scalar.activation(out=v_own[j][:, 0:NB],
                                         in_=ps8[jj * 2], func=AF.Identity)
                    nc.vector.tensor_copy(out=v_own[j][:, NB:2 * NB],
                                          in_=ps8[jj * 2 + 1])
                    nc.sync.dma_start(out=v_send[j * P:(j + 1) * P, :],
                                      in_=v_own[j])
            nc.gpsimd.collective_compute(
                "AllGather", mybir.AluOpType.bypass,
                replica_groups=GROUPS,
                ins=[v_send[:, :]],
                outs=[v_recv[:, :]])

            # Partner-half import: rank parity picks the gathered block.
            pid = nc.sync.partition_id()
            parity = pid % 2
            pbase_x = (1 - parity) * (D // 2)
            nc.sync.dma_start(
                out=xt_par,
                in_=x_recv[bass.ds(pbase_x, D // 2), :].rearrange(
                    "(c p) t -> p c t", p=P))
            pbase_v = (1 - parity) * H
            nc.sync.dma_start(
                out=v_par,
                in_=v_recv[bass.ds(pbase_v, H), :].rearrange(
                    "(j p) e -> p j e", p=P))

            # B_own between the exchanges and C_own: needs no DMA data,
            # so neither a slow input stream nor a late collective can
            # stall the PE here.
            for j in range(HT):
                b_group(j)

            for p in range(4):
                c_pass(p, 0)
            for j in range(HT, ST):
                b_group(j)
            nc.vector.tensor_copy(out=sumacc_bf, in_=sumacc)
            sums_sb = phb.tile([1, Q], F32, tag="sums_sb", name="sums_sb")
            av_saved = None
            for p in range(4):
                ps_o = c_pass(p, 1, last=(p == 3))
                if p == 1:
                    av_saved = ps_o
                if p == 2:
                    # Softmax denominators: two cheap bf16 matmuls into row 0
                    # of pass-1's retired av banks; copies via the ScalarE.
                    for qb in range(2):
                        fs = av_saved[2 + qb][0:1, :]
                        nc.tensor.matmul(
                            fs, ones_b, sumacc_bf[:, qb * NB:(qb + 1) * NB],
                            start=True, stop=True)
                        nc.scalar.activation(
                            out=sums_sb[:, qb * NB:(qb + 1) * NB], in_=fs,
                            func=AF.Identity)
                    nc.sync.dma_start(out=sums[:, :], in_=sums_sb)

    return nc


_NC_CACHE = None


def _get_nc():
    global _NC_CACHE
    if _NC_CACHE is None:
        _NC_CACHE = build_nc()
    return _NC_CACHE


# ---------------------------------------------------------------------------
# Host side
# ---------------------------------------------------------------------------


def _pack_chunks(a):
    """[1024, 1024] (d, cols) -> [512, 2048] 2-d-tile chunk layout."""
    return np.ascontiguousarray(
        a.reshape(4, 2, 128, a.shape[1]).transpose(0, 2, 1, 3)
        .reshape(512, 2 * a.shape[1]))


def _prep_in_maps(x, W_qkv, b_qkv, W_proj, b_proj):
    x = np.asarray(x, dtype=np.float32)
    W_qkv = np.asarray(W_qkv, dtype=np.float32)
    b_qkv = np.asarray(b_qkv, dtype=np.float32)
    W_proj = np.asarray(W_proj, dtype=np.float32)
    b_proj = np.asarray(b_proj, dtype=np.float32)

    scale = 1.0 / math.sqrt(D)
    bf = ml_dtypes.bfloat16
    Wq = W_qkv[:D]
    Wk = W_qkv[D:2 * D]
    Wv = W_qkv[2 * D:]
    b_q = b_qkv[:D]
    b_v = b_qkv[2 * D:]

    M = (Wq.T * scale) @ Wk                    # [d, d']
    wvp = (W_proj @ Wv).T                      # [d, e]
    u = scale * (Wk.T @ b_q)                   # [d]
    b_eff = b_proj + W_proj @ b_v

    m_pack = _pack_chunks(M)
    # split by e'-half so Q' sweep 1's columns stream after sweep 0's
    m0_h = np.ascontiguousarray(np.concatenate(
        [m_pack[:, 0:NB], m_pack[:, D:D + NB]], axis=1)).astype(bf)
    m1_h = np.ascontiguousarray(np.concatenate(
        [m_pack[:, NB:D], m_pack[:, D + NB:2 * D]], axis=1)).astype(bf)
    wvp_h = _pack_chunks(wvp).astype(bf)

    in_maps = []
    for c in range(N_CORES):
        b, h = divmod(c, 2)
        xt_h = _pack_chunks(
            np.ascontiguousarray(x[b, h * H:(h + 1) * H, :].T)).astype(bf)
        beta_all = x[b] @ u                    # [2048] per-key bias
        beta_c = np.concatenate(
            [beta_all[h * H:(h + 1) * H],
             beta_all[(1 - h) * H:(2 - h) * H]]).reshape(ST, P).T
        in_maps.append({"xt": xt_h, "m0": m0_h, "m1": m1_h, "wvp": wvp_h,
                        "beta": np.ascontiguousarray(beta_c,
                                                     dtype=np.float32)})
    return in_maps, b_eff


def _postprocess(results, b_eff):
    y = np.empty((4, S, D), dtype=np.float32)
    for c in range(N_CORES):
        b, h = divmod(c, 2)
        ytc = results[c]["yt"].astype(np.float32)  # [D(e), Q] unnormalized
        sc = results[c]["sums"][0]                 # [Q] softmax denominators
        y[b, h * Q:(h + 1) * Q, :] = ytc.T / sc[:, None] + b_eff[None, :]
    return y


def kernel(x, W_qkv, b_qkv, W_proj, b_proj, **run_kwargs):
    nc = _get_nc()
    in_maps, b_eff = _prep_in_maps(x, W_qkv, b_qkv, W_proj, b_proj)
    last_exc = None
    for attempt in range(3):
        try:
            res = run_bass_kernel_spmd(nc, in_maps,
                                       core_ids=list(range(N_CORES)),
                                       **run_kwargs)
            break
        except Exception as exc:  # transient NRT device errors
            last_exc = exc
            import time
            time.sleep(2.0 * (attempt + 1))
    else:
        raise last_exc
    y = _postprocess(res.results, b_eff)
    kernel.last_result = res
    return y


# revision 39
# speedup vs baseline: 1.0035x; 1.0035x over previous
"""Trainium2 Bass kernel for a single-head attention layer (folded form).

Problem: x [4, 2048, 1024] f32; torch-Linear qkv (W_qkv [3072, 1024]) ->
single-head attention (d=1024) -> output projection (W_proj [1024, 1024]).

Algebraic folds (exact, host-side fp32):
  scores*scale = q.k*scale = x (scale Wq^T Wk) x^T + beta_j + const(query)
    with beta = x @ (scale Wk^T b_q)  (per-key activation bias; the
    query-constant term and the k-bias term cancel in softmax)
  y = (A V) Wp^T = A (x (Wp Wv)^T)  -- proj folded into the V weights.
So the device only computes:
  Q' = x M          (M = scale Wq^T Wk, 128 MMs)
  V'' = x Wvp^T     (Wvp = Wp Wv, 128 MMs)
  S^T = x^T-blocks . Q'^T -> exp(.+beta)   (256 MMs)
  y^T = V''^T-blocks . exp  (256 MMs)
768 N=512 matmuls/core vs 1024 for the direct form: the K^T projection and
the output projection phases are gone entirely.

Sharding: 8 NeuronCores = 4 batches x 2 token-halves. Keys/values for the
partner half arrive via two pairwise AllGathers (replica groups
[[0,1],..]): one 2MB gather of the raw x^T (B's partner stationary data,
mirrored SBUF->DRAM right after the input load since collectives cannot
read IO tensors), and one 2MB gather of V''. The pairwise CC stream is
serial (start barrier, then ops back-to-back), so few large ops keep the
worst-case chain short when peer launch skew delays the barrier.
Exchanged data is consumed as late as possible (B_own -> C_own ->
B_partner -> C_partner) so peer launch skew + collective jitter stays
hidden under compute.

Performance structure (steady state ~216ns per N=512 bf16 matmul):
  - Q' and V'' run as two d-major 8-bank sweeps each, consuming their
    weight chunks in DMA arrival order; 46 dummy warmup matmuls bridge
    the PE to the input-DMA gate (first (m0,x) chunk pair ~14us). The m
    matrix is split into per-sweep column halves (m0/m1) so sweep 0 only
    gates on 3MB of stream.
  - Phase order Q' -> V'' -> B_own -> C_own -> B_par -> C_par: V''
    right after Q' maximizes peer-skew tolerance (the partner's V''
    gather can lag ~80us before C_par feels it), while B_own (which
    needs no input bytes at all) buffers C_own from any stream jitter.
  - All input DMAs are issued on the Sync queue in consumption order
    (m0/x interleaved with m1's first chunk hoisted, then beta, m1, wvp).
    One queue sustains ~300GB/s early / ~420GB/s late; a second queue
    de-pairs arrivals (tried, slower), and 512KB descriptors stream
    measurably faster than 256KB halves (200-260GB/s), so x chunks stay
    whole even though that delays the first matmul slightly.
  - PSUM pools span the whole kernel; accumulation-bank pairs rotate
    [s, av01, s, av23] so a bank is reused only every 4th group.
  - The final C pass is bank-major and its last bank drains in two
    256-col pieces on two DMA queues, so the exposed tail is one
    quarter-add plus one small DMA's completion latency.
  - PSUM->SBUF drains split across ScalarE and VectorE; y^T leaves as
    bf16 (host normalizes in f32).

Measured: 184.3-185.9us typical, rare peer-skew outliers to ~195-220us
when another core's NEFF launches ~40-60us late (the collective start
barrier absorbs the skew; consumption order caps the damage). (vs
242.6us for the direct-form baseline; bf16 matmul
roofline for the folded form is ~164us/core, plus ~7.7us framework
preamble, ~5.7us input-DMA gate, ~3.2us hardware duty-cycle throttle
(432ns every ~10.7us, drifts across program structure - timer-based),
~2.5us ldweights handoff overhead (216.6 vs 213.3ns/MM; walrus
ldw-opt is incompatible with framework-emitted InstLdweights), and
~3us output-DMA tail.
"""

import math

import numpy as np
import ml_dtypes

import concourse.bass as bass
import concourse.tile as tile
from concourse import mybir
from concourse.bass_utils import run_bass_kernel_spmd
from concourse.vector_clock import ScopedClock, VectorClock

BF16 = mybir.dt.bfloat16
F32 = mybir.dt.float32
AF = mybir.ActivationFunctionType

D = 1024   # model dim
S = 2048   # sequence length
Q = 1024   # queries per core
H = 1024   # keys per core (own half)
P = 128    # SBUF partitions
NB = 512   # matmul moving-block size
DT = D // P
HT = H // P
ST = S // P
N_CORES = 8
GROUPS = [[0, 1], [2, 3], [4, 5], [6, 7]]
WARMUP_MMS = 46  # bridges the PE to the input-DMA gate (first d-tile pair)

# ---------------------------------------------------------------------------
# Workarounds for this container's walrus, which rejects any instruction
# carrying more than one sem wait ("Too many sync wait commands").
# ---------------------------------------------------------------------------


def _patched_drain_and_barrier(self, tick_clock, wait_clock):
    # Split the kernel-tail drain into one drain per semaphore (1 wait each).
    gc = tick_clock.global_clock
    n = len(gc)
    for i in range(n):
        if gc[i] > 0:
            vec = [0] * n
            vec[i] = gc[i]
            dr = self.nc.sync.drain()
            wait_clock.add_sem_waits(dr.ins, ScopedClock({None: VectorClock(vec)}))
    # sem-only barriers: the per-sem drains above already imply all
    # sem-updating work has completed; skip the per-engine InstDrains.
    self.nc.all_engine_barrier(sem_only=True)
    popped = self.nc._tile_sem_poison_stack.pop()
    assert popped is self._sem_poison
    self.nc.clear_and_free_semaphores(list(self.sems.allocated().values()))
    self.nc.all_engine_barrier(sem_only=True)


_MAX_WAITS = 1
_split_counter = [0]


def _split_excess_waits(ordered):
    # Hoist excess waits onto preceding same-engine NoOps.
    for insts in ordered.values():
        new_list = []
        for inst in insts:
            si = inst.sync_info
            waits = list(si.on_wait) if si is not None and si.on_wait else []
            if len(waits) > _MAX_WAITS and inst.engine is not None:
                extra, keep = waits[:-_MAX_WAITS], waits[-_MAX_WAITS:]
                for w in extra:
                    _split_counter[0] += 1
                    nop = mybir.InstNoOp(
                        name=f"waitsplit-{_split_counter[0]}",
                        sync_info=mybir.SyncInfo(on_wait=[w], on_update=[]),
                        bass_nofuse=True,
                        engine=inst.engine,
                    )
                    new_list.append(nop)
                inst.sync_info = mybir.SyncInfo(
                    on_wait=keep, on_update=list(si.on_update))
            new_list.append(inst)
        insts[:] = new_list


def _install_patches():
    if getattr(tile.TileContext, "_attn_patched", False):
        return
    tile.TileContext._drain_and_barrier = _patched_drain_and_barrier
    orig_lower = tile.TileContext._lower_ordered_insts

    def _lower_with_wait_split(self, ordered):
        _split_excess_waits(ordered)
        return orig_lower(self, ordered)

    tile.TileContext._lower_ordered_insts = _lower_with_wait_split
    tile.TileContext._attn_patched = True


_install_patches()

# ---------------------------------------------------------------------------
# Device program
# ---------------------------------------------------------------------------


def build_nc():
    nc = bass.Bass("TRN2", target_bir_lowering=False, debug=False,
                   num_devices=N_CORES)

    # All weight-like tensors are packed partition-major in 2-d-tile chunks
    # (rows r = 128*b + p hold d-tiles (2b, 2b+1) side by side): 4KB DMA
    # lines, and d-major sweeps consume chunks in arrival order.
    xt = nc.dram_tensor("xt", [D // 2, 2 * H], BF16, kind="ExternalInput").ap()
    m0 = nc.dram_tensor("m0", [D // 2, D], BF16, kind="ExternalInput").ap()
    m1 = nc.dram_tensor("m1", [D // 2, D], BF16, kind="ExternalInput").ap()
    wvp = nc.dram_tensor("wvp", [D // 2, 2 * D], BF16,
                         kind="ExternalInput").ap()
    beta = nc.dram_tensor("beta", [P, ST], F32, kind="ExternalInput").ap()
    yt = nc.dram_tensor("yt", [D, Q], BF16, kind="ExternalOutput").ap()
    sums = nc.dram_tensor("sums", [1, Q], F32, kind="ExternalOutput").ap()

    x_mir = nc.dram_tensor("x_mir", [D // 2, 2 * H], BF16).ap()
    x_recv = nc.dram_tensor("x_recv", [D, 2 * H], BF16).ap()
    v_send = nc.dram_tensor("v_send", [H, D], BF16).ap()
    v_recv = nc.dram_tensor("v_recv", [2 * H, D], BF16).ap()

    from contextlib import ExitStack
    with tile.TileContext(nc) as tc, ExitStack() as stack:
        res = stack.enter_context(tc.tile_pool(name="res", bufs=1))
        # x^T own chunks persist into phase B (B_own stationary data).
        xch = [res.tile([P, 2 * H], BF16, tag=f"x{b}", name=f"x{b}")
               for b in range(DT // 2)]
        qt_t = [res.tile([P, Q], BF16, tag=f"qt{e}", name=f"qt{e}")
                for e in range(DT)]
        v_own = [res.tile([P, D], BF16, tag=f"vo{j}", name=f"vo{j}")
                 for j in range(HT)]
        beta_t = res.tile([P, ST], F32, tag="beta", name="beta")

        # PSUM pools span the whole kernel: closing a PSUM pool inserts a
        # full PE-drain barrier before the next phase can reuse the banks.
        scr_ps = stack.enter_context(
            tc.tile_pool(name="scr_ps", bufs=2, space="PSUM"))
        av_ps = stack.enter_context(
            tc.tile_pool(name="av_ps", bufs=1, space="PSUM"))
        pair_cycle = [0]

        def next_pair(nm):
            # Rotate accumulation-bank pairs [s, av01, s, av23] so a bank is
            # reused only every 4th group - its PSUM->SBUF drain is long done.
            mmod = pair_cycle[0] % 4
            pair_cycle[0] += 1
            if mmod == 1:
                tags = ("av0", "av1")
                pool = av_ps
            elif mmod == 3:
                tags = ("av2", "av3")
                pool = av_ps
            else:
                tags = ("s0", "s1")
                pool = scr_ps
            return [pool.tile([P, NB], F32, tag=t, name=f"{nm}_{i}")
                    for i, t in enumerate(tags)]

        def sweep8(nm):
            return ([scr_ps.tile([P, NB], F32, tag=t, name=f"{nm}_{i}")
                     for i, t in enumerate(("s0", "s1", "s0", "s1"))]
                    + [av_ps.tile([P, NB], F32, tag=f"av{i}",
                                  name=f"{nm}_av{i}") for i in range(4)])

        # ---------------- Phase A: projections + exchanges ----------------
        if True:
            mch = [[res.tile([P, D], BF16, tag=f"m{s}{b}", name=f"m{s}{b}")
                    for b in range(DT // 2)] for s in range(2)]
            wch = [res.tile([P, 2 * D], BF16, tag=f"wvp{b}", name=f"wvp{b}")
                   for b in range(DT // 2)]

            def xt_sl(d, sl):
                base = (d % 2) * H
                return xch[d // 2][:, base + sl.start:base + sl.stop]

            def m_sl(s, d, sl):
                # sweep-half s, d-tile d, e'-columns sl within the half
                base = (d % 2) * (D // 2)
                return mch[s][d // 2][:, base + sl.start:base + sl.stop]

            def wvp_sl(d, sl):
                base = (d % 2) * D
                return wch[d // 2][:, base + sl.start:base + sl.stop]

            warm = res.tile([P, P], BF16, tag="warm", name="warm")
            nc.vector.memset(warm, 0.125)

            # All input DMAs on the Sync queue in consumption order: the
            # queue transfers FIFO (two queues were tried and are slower -
            # the early DMA path is a shared bottleneck and interleaving
            # de-pairs the m/x arrivals); wvp streams after the m/x chunks,
            # well before V''. The first chunks are split at d-tile
            # granularity so Q' can start half a chunk earlier.
            for b in range(DT // 2):
                nc.sync.dma_start(out=mch[0][b], in_=m0[b * P:(b + 1) * P, :])
                # whole 512KB x chunks: large descriptors sustain a visibly
                # higher early stream rate than 256KB halves
                nc.sync.dma_start(out=xch[b], in_=xt[b * P:(b + 1) * P, :])
            nc.sync.dma_start(out=beta_t, in_=beta[:, :])
            for b in range(DT // 2):
                nc.sync.dma_start(out=mch[1][b], in_=m1[b * P:(b + 1) * P, :])
            for b in range(DT // 2):
                nc.sync.dma_start(out=wch[b], in_=wvp[b * P:(b + 1) * P, :])

            # Dummy matmuls on the memset tile: keep the PE busy while the
            # first m/x chunks stream in so HAM reaches K=8/8.
            wm_ps = scr_ps.tile([P, NB], F32, tag="s0", name="wm")
            for i in range(WARMUP_MMS):
                nc.tensor.matmul(wm_ps[:, 0:P], warm, warm,
                                 start=(i == 0), stop=(i == WARMUP_MMS - 1))

            # Q'^T[e, q] in two d-major 8-bank sweeps of 4 e-tiles x 2
            # halves, so the PE consumes m[d]/xt[d] in DMA arrival order
            # instead of stalling a whole e-group on the last d-tile.
            for s in range(2):
                ps8 = sweep8(f"qp{s}")
                for d in range(DT):
                    for i in range(4):
                        e = s * 4 + i
                        for h in range(2):
                            nc.tensor.matmul(
                                ps8[i * 2 + h],
                                m_sl(s, d, slice(i * P, (i + 1) * P)),
                                xt_sl(d, slice(h * NB, (h + 1) * NB)),
                                start=(d == 0), stop=(d == DT - 1))
                for i in range(4):
                    e = s * 4 + i
                    nc.scalar.activation(
                        out=qt_t[e][:, 0:NB], in_=ps8[i * 2],
                        func=AF.Identity)
                    nc.vector.tensor_copy(
                        out=qt_t[e][:, NB:2 * NB], in_=ps8[i * 2 + 1])

            # x^T exchange: collectives cannot read IO tensors, so mirror
            # the loaded chunks SBUF->DRAM on the Scalar queue. Issued
            # after the Q' sweeps so the 2MB of mirror writes never touch
            # HBM during the input-stream gate; partner x^T is consumed
            # last (B_partner), leaving ~90us of slack.
            for b in range(DT // 2):
                nc.scalar.dma_start(out=x_mir[b * P:(b + 1) * P, :],
                                    in_=xch[b])
            # One merged 2MB gather: the pairwise CC stream is SERIAL
            # (barrier, then each op back-to-back at ~10-17us apiece), so
            # fewer, larger ops shorten the worst-case chain when peer
            # launch skew delays the stream's start barrier.
            nc.gpsimd.collective_compute(
                "AllGather", mybir.AluOpType.bypass,
                replica_groups=GROUPS,
                ins=[x_mir[:, :]],
                outs=[x_recv[:, :]])

        # ---------------- Phases B, C ----------------
        if True:
            phb = res
            # Partner-half import tiles (filled after V'' below).
            xt_par = phb.tile([P, DT // 2, 2 * H], BF16, tag="xp", name="xp")
            v_par = phb.tile([P, HT, D], BF16, tag="vp", name="vp")

            exp_t = [phb.tile([P, Q], BF16, tag=f"exp{j}", name=f"exp{j}")
                     for j in range(ST)]
            y_acc = [phb.tile([P, Q], BF16, tag=f"ya{e}", name=f"ya{e}")
                     for e in range(DT)]

            # f32 per-key-lane partial sums, accumulated on the (idle) DVE;
            # cast to bf16 and reduced across partitions with two cheap
            # bf16 matmuls.
            sumacc = phb.tile([P, Q], F32, tag="sumacc", name="sumacc")
            sumacc_bf = phb.tile([P, Q], BF16, tag="sumbf", name="sumbf")
            ones_b = phb.tile([P, 1], BF16, tag="ones_b", name="ones_b")
            nc.vector.memset(ones_b, 1.0)

            def xkey_sl(e, j):
                # stationary [128 d-rows of e-block, 128 key-tokens]
                if j < HT:
                    return xt_sl(e, slice(j * P, (j + 1) * P))
                jj = j - HT
                base = (e % 2) * H
                return xt_par[:, e // 2, base + jj * P:base + (jj + 1) * P]

            def v_tile(j):
                if j < HT:
                    return v_own[j]
                return v_par[:, j - HT, :]

            # Exchanged data is consumed as late as possible:
            #   B_own -> C_own (own keys only) -> B_partner -> C_partner.
            def b_group(j):
                ps = next_pair(f"bps{j}")
                for e in range(DT):
                    for qb in range(2):
                        nc.tensor.matmul(
                            ps[qb], xkey_sl(e, j),
                            qt_t[e][:, qb * NB:(qb + 1) * NB],
                            start=(e == 0), stop=(e == DT - 1))
                for qb in range(2):
                    sl = slice(qb * NB, (qb + 1) * NB)
                    nc.scalar.activation(
                        out=exp_t[j][:, sl], in_=ps[qb], func=AF.Exp,
                        bias=beta_t[:, j:j + 1])
                    if j == 0:
                        nc.vector.tensor_copy(
                            out=sumacc[:, sl], in_=exp_t[j][:, sl])
                    else:
                        nc.vector.tensor_add(
                            sumacc[:, sl], sumacc[:, sl], exp_t[j][:, sl])

            def c_pass(p, half, last=False):
                # 2 e-tiles x 2 q-halves = 4 PSUM banks, s/av sets
                # alternating so a pass never waits on the previous drain.
                # half 0 writes y_acc; half 1 accumulates into it (DVE) and
                # the finished y^T tiles stream out.
                if p % 2 == 0:
                    ps_o = [scr_ps.tile([P, NB], F32, tag=f"s{i % 2}",
                                        name=f"cps{half}_{p}_{i}")
                            for i in range(4)]
                else:
                    ps_o = [av_ps.tile([P, NB], F32, tag=f"av{i}",
                                       name=f"cps{half}_{p}_{i}")
                            for i in range(4)]
                j0 = half * HT

                def mms(i, qb, j):
                    nc.tensor.matmul(
                        ps_o[i * 2 + qb],
                        v_tile(j)[:, (p * 2 + i) * P:(p * 2 + i + 1) * P],
                        exp_t[j][:, qb * NB:(qb + 1) * NB],
                        start=(j == j0), stop=(j == j0 + HT - 1))

                def drain(i, qb):
                    dp = p * 2 + i
                    sl = slice(qb * NB, (qb + 1) * NB)
                    if half == 0:
                        if qb == 0:
                            nc.scalar.activation(out=y_acc[dp][:, sl],
                                                 in_=ps_o[i * 2],
                                                 func=AF.Identity)
                        else:
                            nc.vector.tensor_copy(out=y_acc[dp][:, sl],
                                                  in_=ps_o[i * 2 + 1])
                    elif last and i == 1 and qb == 1:
                        # Final bank: drain in two 256-col pieces on two DMA
                        # queues so the exposed tail is one quarter-add +
                        # two overlapped small DMAs.
                        for t in range(2):
                            tsl = slice(qb * NB + t * (NB // 2),
                                        qb * NB + (t + 1) * (NB // 2))
                            psl = slice(t * (NB // 2), (t + 1) * (NB // 2))
                            nc.vector.tensor_add(
                                y_acc[dp][:, tsl], y_acc[dp][:, tsl],
                                ps_o[i * 2 + qb][:, psl])
                            eng = nc.scalar if t == 0 else nc.sync
                            eng.dma_start(
                                out=yt[dp * P:(dp + 1) * P, tsl],
                                in_=y_acc[dp][:, tsl])
                    else:
                        nc.vector.tensor_add(
                            y_acc[dp][:, sl], y_acc[dp][:, sl],
                            ps_o[i * 2 + qb])
                        eng = nc.scalar if qb == 0 else nc.sync
                        eng.dma_start(out=yt[dp * P:(dp + 1) * P, sl],
                                      in_=y_acc[dp][:, sl])

                if last:
                    # Bank-major: each bank's 8-key-tile accumulation
                    # finishes 8 matmul slots before the next one, so its
                    # DVE add + y^T DMA overlap the remaining matmuls and
                    # only the final bank's drain is exposed in the tail.
                    for i in range(2):
                        for qb in range(2):
                            for j in range(j0, j0 + HT):
                                mms(i, qb, j)
                            drain(i, qb)
                else:
                    for j in range(j0, j0 + HT):
                        for i in range(2):
                            for qb in range(2):
                                mms(i, qb, j)
                    for i in range(2):
                        for qb in range(2):
                            drain(i, qb)
                return ps_o

            # V''[token, e] -> SBUF (kept) + v_send; AllGather per half.
            # Two d-major 8-bank sweeps (4 token-tiles x 2 e-halves) so the
            # PE consumes wvp[d] in DMA arrival order - sweep 0 starts as
            # soon as wvp chunk 0 lands instead of stalling on the last one.
            for s in range(2):
                ps8 = sweep8(f"vp{s}")
                for d in range(DT):
                    for jj in range(4):
                        j = s * 4 + jj
                        for eb in range(2):
                            nc.tensor.matmul(
                                ps8[jj * 2 + eb],
                                xt_sl(d, slice(j * P, (j + 1) * P)),
                                wvp_sl(d, slice(eb * NB, (eb + 1) * NB)),
                                start=(d == 0), stop=(d == DT - 1))
                for jj in range(4):
                    j = s * 4 + jj
                    nc.scalar.activation(out=v_own[j][:, 0:NB],
                                         in_=ps8[jj * 2], func=AF.Identity)
                    nc.vector.tensor_copy(out=v_own[j][:, NB:2 * NB],
                                          in_=ps8[jj * 2 + 1])
                    nc.sync.dma_start(out=v_send[j * P:(j + 1) * P, :],
                                      in_=v_own[j])
            nc.gpsimd.collective_compute(
                "AllGather", mybir.AluOpType.bypass,
                replica_groups=GROUPS,
                ins=[v_send[:, :]],
                outs=[v_recv[:, :]])

            # Partner-half import: rank parity picks the gathered block.
            pid = nc.sync.partition_id()
            parity = pid % 2
            pbase_x = (1 - parity) * (D // 2)
            nc.sync.dma_start(
                out=xt_par,
                in_=x_recv[bass.ds(pbase_x, D // 2), :].rearrange(
                    "(c p) t -> p c t", p=P))
            pbase_v = (1 - parity) * H
            nc.sync.dma_start(
                out=v_par,
                in_=v_recv[bass.ds(pbase_v, H), :].rearrange(
                    "(j p) e -> p j e", p=P))

            # B_own between the exchanges and C_own: needs no DMA data,
            # so neither a slow input stream nor a late collective can
            # stall the PE here.
            for j in range(HT):
                b_group(j)

            for p in range(4):
                c_pass(p, 0)
            for j in range(HT, ST):
                b_group(j)
            nc.vector.tensor_copy(out=sumacc_bf, in_=sumacc)
            sums_sb = phb.tile([1, Q], F32, tag="sums_sb", name="sums_sb")
            av_saved = None
            for p in range(4):
                ps_o = c_pass(p, 1, last=(p == 3))
                if p == 1:
                    av_saved = ps_o
                if p == 2:
                    # Softmax denominators: two cheap bf16 matmuls into row 0
                    # of pass-1's retired av banks; copies via the ScalarE.
                    for qb in range(2):
                        fs = av_saved[2 + qb][0:1, :]
                        nc.tensor.matmul(
                            fs, ones_b, sumacc_bf[:, qb * NB:(qb + 1) * NB],
                            start=True, stop=True)
                        nc.scalar.activation(
                            out=sums_sb[:, qb * NB:(qb + 1) * NB], in_=fs,
                            func=AF.Identity)
                    nc.sync.dma_start(out=sums[:, :], in_=sums_sb)

    return nc


_NC_CACHE = None


def _get_nc():
    global _NC_CACHE
    if _NC_CACHE is None:
        _NC_CACHE = build_nc()
    return _NC_CACHE


# ---------------------------------------------------------------------------
# Host side
# ---------------------------------------------------------------------------


def _pack_chunks(a):
    """[1024, 1024] (d, cols) -> [512, 2048] 2-d-tile chunk layout."""
    return np.ascontiguousarray(
        a.reshape(4, 2, 128, a.shape[1]).transpose(0, 2, 1, 3)
        .reshape(512, 2 * a.shape[1]))


def _prep_in_maps(x, W_qkv, b_qkv, W_proj, b_proj):
    x = np.asarray(x, dtype=np.float32)
    W_qkv = np.asarray(W_qkv, dtype=np.float32)
    b_qkv = np.asarray(b_qkv, dtype=np.float32)
    W_proj = np.asarray(W_proj, dtype=np.float32)
    b_proj = np.asarray(b_proj, dtype=np.float32)

    scale = 1.0 / math.sqrt(D)
    bf = ml_dtypes.bfloat16
    Wq = W_qkv[:D]
    Wk = W_qkv[D:2 * D]
    Wv = W_qkv[2 * D:]
    b_q = b_qkv[:D]
    b_v = b_qkv[2 * D:]

    M = (Wq.T * scale) @ Wk                    # [d, d']
    wvp = (W_proj @ Wv).T                      # [d, e]
    u = scale * (Wk.T @ b_q)                   # [d]
    b_eff = b_proj + W_proj @ b_v

    m_pack = _pack_chunks(M)
    # split by e'-half so Q' sweep 1's columns stream after sweep 0's
    m0_h = np.ascontiguousarray(np.concatenate(
        [m_pack[:, 0:NB], m_pack[:, D:D + NB]], axis=1)).astype(bf)
    m1_h = np.ascontiguousarray(np.concatenate(
        [m_pack[:, NB:D], m_pack[:, D + NB:2 * D]], axis=1)).astype(bf)
    wvp_h = _pack_chunks(wvp).astype(bf)

    in_maps = []
    for c in range(N_CORES):
        b, h = divmod(c, 2)
        xt_h = _pack_chunks(
            np.ascontiguousarray(x[b, h * H:(h + 1) * H, :].T)).astype(bf)
        beta_all = x[b] @ u                    # [2048] per-key bias
        beta_c = np.concatenate(
            [beta_all[h * H:(h + 1) * H],
             beta_all[(1 - h) * H:(2 - h) * H]]).reshape(ST, P).T
        in_maps.append({"xt": xt_h, "m0": m0_h, "m1": m1_h, "wvp": wvp_h,
                        "beta": np.ascontiguousarray(beta_c,
                                                     dtype=np.float32)})
    return in_maps, b_eff


def _postprocess(results, b_eff):
    y = np.empty((4, S, D), dtype=np.float32)
    for c in range(N_CORES):
        b, h = divmod(c, 2)
        ytc = results[c]["yt"].astype(np.float32)  # [D(e), Q] unnormalized
        sc = results[c]["sums"][0]                 # [Q] softmax denominators
        y[b, h * Q:(h + 1) * Q, :] = ytc.T / sc[:, None] + b_eff[None, :]
    return y


def kernel(x, W_qkv, b_qkv, W_proj, b_proj, **run_kwargs):
    nc = _get_nc()
    in_maps, b_eff = _prep_in_maps(x, W_qkv, b_qkv, W_proj, b_proj)
    last_exc = None
    for attempt in range(3):
        try:
            res = run_bass_kernel_spmd(nc, in_maps,
                                       core_ids=list(range(N_CORES)),
                                       **run_kwargs)
            break
        except Exception as exc:  # transient NRT device errors
            last_exc = exc
            import time
            time.sleep(2.0 * (attempt + 1))
    else:
        raise last_exc
    y = _postprocess(res.results, b_eff)
    kernel.last_result = res
    return y


# revision 40
# speedup vs baseline: 1.0133x; 1.0098x over previous
"""Trainium2 Bass kernel for a single-head attention layer (folded form).

Problem: x [4, 2048, 1024] f32; torch-Linear qkv (W_qkv [3072, 1024]) ->
single-head attention (d=1024) -> output projection (W_proj [1024, 1024]).

Algebraic folds (exact, host-side fp32):
  scores*scale = q.k*scale = x (scale Wq^T Wk) x^T + beta_j + const(query)
    with beta = x @ (scale Wk^T b_q)  (per-key activation bias; the
    query-constant term and the k-bias term cancel in softmax)
  y = (A V) Wp^T = A (x (Wp Wv)^T)  -- proj folded into the V weights.
So the device only computes:
  Q' = x M          (M = scale Wq^T Wk, 128 MMs)
  V'' = x Wvp^T     (Wvp = Wp Wv, 128 MMs)
  S^T = x^T-blocks . Q'^T -> exp(.+beta)   (256 MMs)
  y^T = V''^T-blocks . exp  (256 MMs)
768 N=512 matmuls/core vs 1024 for the direct form: the K^T projection and
the output projection phases are gone entirely.

Sharding: 8 NeuronCores = 4 batches x 2 token-halves. Keys/values for the
partner half arrive via two pairwise AllGathers (replica groups
[[0,1],..]): one 2MB gather of the raw x^T (B's partner stationary data,
mirrored SBUF->DRAM right after the input load since collectives cannot
read IO tensors), and one 2MB gather of V''. The pairwise CC stream is
serial (start barrier, then ops back-to-back), so few large ops keep the
worst-case chain short when peer launch skew delays the barrier.
Exchanged data is consumed as late as possible (B_own -> C_own ->
B_partner -> C_partner) so peer launch skew + collective jitter stays
hidden under compute.

Performance structure (steady state ~216ns per N=512 bf16 matmul):
  - Q' and V'' run as two d-major 8-bank sweeps each, consuming their
    weight chunks in DMA arrival order; 46 dummy warmup matmuls bridge
    the PE to the input-DMA gate (first (m0,x) chunk pair ~14us). The m
    matrix is split into per-sweep column halves (m0/m1) so sweep 0 only
    gates on 3MB of stream.
  - Phase order Q' -> V'' -> B_own -> C_own -> B_par -> C_par: V''
    right after Q' maximizes peer-skew tolerance (the partner's V''
    gather can lag ~80us before C_par feels it), while B_own (which
    needs no input bytes at all) buffers C_own from any stream jitter.
  - All input DMAs are issued on the Sync queue in consumption order
    (m0/x interleaved, then beta, m1, wvp).
    One queue sustains ~300GB/s early / ~420GB/s late; a second queue
    de-pairs arrivals (tried, slower), and 512KB descriptors stream
    measurably faster than 256KB halves (200-260GB/s), so x chunks stay
    whole even though that delays the first matmul slightly.
  - PSUM pools span the whole kernel; accumulation-bank pairs rotate
    [s, av01, s, av23] so a bank is reused only every 4th group.
  - The final C pass is bank-major and its last bank drains in two
    256-col pieces on two DMA queues, so the exposed tail is one
    quarter-add plus one small DMA's completion latency.
  - PSUM->SBUF drains split across ScalarE and VectorE; y^T leaves as
    bf16 (host normalizes in f32).

Measured: 184.3-185.9us typical, rare peer-skew outliers to ~195-220us
when another core's NEFF launches ~40-60us late (the collective start
barrier absorbs the skew; consumption order caps the damage). (vs
242.6us for the direct-form baseline; bf16 matmul
roofline for the folded form is ~164us/core, plus ~7.7us framework
preamble, ~5.7us input-DMA gate, ~3.2us hardware duty-cycle throttle
(432ns every ~10.7us, drifts across program structure - timer-based),
~2.5us ldweights handoff overhead (216.6 vs 213.3ns/MM; walrus
ldw-opt is incompatible with framework-emitted InstLdweights), and
~3us output-DMA tail.
"""

import math

import numpy as np
import ml_dtypes

import concourse.bass as bass
import concourse.tile as tile
from concourse import mybir
from concourse.bass_utils import run_bass_kernel_spmd
from concourse.vector_clock import ScopedClock, VectorClock

BF16 = mybir.dt.bfloat16
F32 = mybir.dt.float32
AF = mybir.ActivationFunctionType

D = 1024   # model dim
S = 2048   # sequence length
Q = 1024   # queries per core
H = 1024   # keys per core (own half)
P = 128    # SBUF partitions
NB = 512   # matmul moving-block size
DT = D // P
HT = H // P
ST = S // P
N_CORES = 8
GROUPS = [[0, 1], [2, 3], [4, 5], [6, 7]]
WARMUP_MMS = 46  # bridges the PE to the input-DMA gate (first d-tile pair)

# ---------------------------------------------------------------------------
# Workarounds for this container's walrus, which rejects any instruction
# carrying more than one sem wait ("Too many sync wait commands").
# ---------------------------------------------------------------------------


def _patched_drain_and_barrier(self, tick_clock, wait_clock):
    # Split the kernel-tail drain into one drain per semaphore (1 wait each).
    gc = tick_clock.global_clock
    n = len(gc)
    for i in range(n):
        if gc[i] > 0:
            vec = [0] * n
            vec[i] = gc[i]
            dr = self.nc.sync.drain()
            wait_clock.add_sem_waits(dr.ins, ScopedClock({None: VectorClock(vec)}))
    # sem-only barriers: the per-sem drains above already imply all
    # sem-updating work has completed; skip the per-engine InstDrains.
    self.nc.all_engine_barrier(sem_only=True)
    popped = self.nc._tile_sem_poison_stack.pop()
    assert popped is self._sem_poison
    self.nc.clear_and_free_semaphores(list(self.sems.allocated().values()))
    self.nc.all_engine_barrier(sem_only=True)


_MAX_WAITS = 1
_split_counter = [0]


def _split_excess_waits(ordered):
    # Hoist excess waits onto preceding same-engine NoOps.
    for insts in ordered.values():
        new_list = []
        for inst in insts:
            si = inst.sync_info
            waits = list(si.on_wait) if si is not None and si.on_wait else []
            if len(waits) > _MAX_WAITS and inst.engine is not None:
                extra, keep = waits[:-_MAX_WAITS], waits[-_MAX_WAITS:]
                for w in extra:
                    _split_counter[0] += 1
                    nop = mybir.InstNoOp(
                        name=f"waitsplit-{_split_counter[0]}",
                        sync_info=mybir.SyncInfo(on_wait=[w], on_update=[]),
                        bass_nofuse=True,
                        engine=inst.engine,
                    )
                    new_list.append(nop)
                inst.sync_info = mybir.SyncInfo(
                    on_wait=keep, on_update=list(si.on_update))
            new_list.append(inst)
        insts[:] = new_list


def _install_patches():
    if getattr(tile.TileContext, "_attn_patched", False):
        return
    tile.TileContext._drain_and_barrier = _patched_drain_and_barrier
    orig_lower = tile.TileContext._lower_ordered_insts

    def _lower_with_wait_split(self, ordered):
        _split_excess_waits(ordered)
        return orig_lower(self, ordered)

    tile.TileContext._lower_ordered_insts = _lower_with_wait_split
    tile.TileContext._attn_patched = True


_install_patches()

# ---------------------------------------------------------------------------
# Device program
# ---------------------------------------------------------------------------


def build_nc():
    nc = bass.Bass("TRN2", target_bir_lowering=False, debug=False,
                   num_devices=N_CORES)

    # All weight-like tensors are packed partition-major in 2-d-tile chunks
    # (rows r = 128*b + p hold d-tiles (2b, 2b+1) side by side): 4KB DMA
    # lines, and d-major sweeps consume chunks in arrival order.
    xt = nc.dram_tensor("xt", [D // 2, 2 * H], BF16, kind="ExternalInput").ap()
    m0 = nc.dram_tensor("m0", [D // 2, D], BF16, kind="ExternalInput").ap()
    m1 = nc.dram_tensor("m1", [D // 2, D], BF16, kind="ExternalInput").ap()
    wvp = nc.dram_tensor("wvp", [D // 2, 2 * D], BF16,
                         kind="ExternalInput").ap()
    beta = nc.dram_tensor("beta", [P, ST], F32, kind="ExternalInput").ap()
    yt = nc.dram_tensor("yt", [D, Q], BF16, kind="ExternalOutput").ap()
    sums = nc.dram_tensor("sums", [1, Q], F32, kind="ExternalOutput").ap()

    x_mir = nc.dram_tensor("x_mir", [D // 2, 2 * H], BF16).ap()
    x_recv = nc.dram_tensor("x_recv", [D, 2 * H], BF16).ap()
    v_send = nc.dram_tensor("v_send", [H, D], BF16).ap()
    v_recv = nc.dram_tensor("v_recv", [2 * H, D], BF16).ap()

    from contextlib import ExitStack
    with tile.TileContext(nc) as tc, ExitStack() as stack:
        res = stack.enter_context(tc.tile_pool(name="res", bufs=1))
        # x^T own chunks persist into phase B (B_own stationary data).
        xch = [res.tile([P, 2 * H], BF16, tag=f"x{b}", name=f"x{b}")
               for b in range(DT // 2)]
        qt_t = [res.tile([P, Q], BF16, tag=f"qt{e}", name=f"qt{e}")
                for e in range(DT)]
        v_own = [res.tile([P, D], BF16, tag=f"vo{j}", name=f"vo{j}")
                 for j in range(HT)]
        beta_t = res.tile([P, ST], F32, tag="beta", name="beta")

        # PSUM pools span the whole kernel: closing a PSUM pool inserts a
        # full PE-drain barrier before the next phase can reuse the banks.
        scr_ps = stack.enter_context(
            tc.tile_pool(name="scr_ps", bufs=2, space="PSUM"))
        av_ps = stack.enter_context(
            tc.tile_pool(name="av_ps", bufs=1, space="PSUM"))
        pair_cycle = [0]

        def next_pair(nm):
            # Rotate accumulation-bank pairs [s, av01, s, av23] so a bank is
            # reused only every 4th group - its PSUM->SBUF drain is long done.
            mmod = pair_cycle[0] % 4
            pair_cycle[0] += 1
            if mmod == 1:
                tags = ("av0", "av1")
                pool = av_ps
            elif mmod == 3:
                tags = ("av2", "av3")
                pool = av_ps
            else:
                tags = ("s0", "s1")
                pool = scr_ps
            return [pool.tile([P, NB], F32, tag=t, name=f"{nm}_{i}")
                    for i, t in enumerate(tags)]

        def sweep8(nm):
            return ([scr_ps.tile([P, NB], F32, tag=t, name=f"{nm}_{i}")
                     for i, t in enumerate(("s0", "s1", "s0", "s1"))]
                    + [av_ps.tile([P, NB], F32, tag=f"av{i}",
                                  name=f"{nm}_av{i}") for i in range(4)])

        # ---------------- Phase A: projections + exchanges ----------------
        if True:
            mch = [[res.tile([P, D], BF16, tag=f"m{s}{b}", name=f"m{s}{b}")
                    for b in range(DT // 2)] for s in range(2)]
            wch = [res.tile([P, 2 * D], BF16, tag=f"wvp{b}", name=f"wvp{b}")
                   for b in range(DT // 2)]

            def xt_sl(d, sl):
                base = (d % 2) * H
                return xch[d // 2][:, base + sl.start:base + sl.stop]

            def m_sl(s, d, sl):
                # sweep-half s, d-tile d, e'-columns sl within the half
                base = (d % 2) * (D // 2)
                return mch[s][d // 2][:, base + sl.start:base + sl.stop]

            def wvp_sl(d, sl):
                base = (d % 2) * D
                return wch[d // 2][:, base + sl.start:base + sl.stop]

            warm = res.tile([P, P], BF16, tag="warm", name="warm")
            nc.vector.memset(warm, 0.125)

            # All input DMAs on the Sync queue in consumption order: the
            # queue transfers FIFO (two queues were tried and are slower -
            # the early DMA path is a shared bottleneck and interleaving
            # de-pairs the m/x arrivals); wvp streams after the m/x chunks,
            # well before V''. The first chunks are split at d-tile
            # granularity so Q' can start half a chunk earlier.
            for b in range(DT // 2):
                nc.sync.dma_start(out=mch[0][b], in_=m0[b * P:(b + 1) * P, :])
                # whole 512KB x chunks: large descriptors sustain a visibly
                # higher early stream rate than 256KB halves
                nc.sync.dma_start(out=xch[b], in_=xt[b * P:(b + 1) * P, :])
            nc.sync.dma_start(out=beta_t, in_=beta[:, :])
            for b in range(DT // 2):
                nc.sync.dma_start(out=mch[1][b], in_=m1[b * P:(b + 1) * P, :])
            for b in range(DT // 2):
                nc.sync.dma_start(out=wch[b], in_=wvp[b * P:(b + 1) * P, :])

            # Dummy matmuls on the memset tile: keep the PE busy while the
            # first m/x chunks stream in so HAM reaches K=8/8.
            wm_ps = scr_ps.tile([P, NB], F32, tag="s0", name="wm")
            for i in range(WARMUP_MMS):
                nc.tensor.matmul(wm_ps[:, 0:P], warm, warm,
                                 start=(i == 0), stop=(i == WARMUP_MMS - 1))

            # Q'^T[e, q] in two d-major 8-bank sweeps of 4 e-tiles x 2
            # halves, so the PE consumes m[d]/xt[d] in DMA arrival order
            # instead of stalling a whole e-group on the last d-tile.
            for s in range(2):
                ps8 = sweep8(f"qp{s}")
                for d in range(DT):
                    for i in range(4):
                        e = s * 4 + i
                        for h in range(2):
                            nc.tensor.matmul(
                                ps8[i * 2 + h],
                                m_sl(s, d, slice(i * P, (i + 1) * P)),
                                xt_sl(d, slice(h * NB, (h + 1) * NB)),
                                start=(d == 0), stop=(d == DT - 1))
                for i in range(4):
                    e = s * 4 + i
                    nc.scalar.activation(
                        out=qt_t[e][:, 0:NB], in_=ps8[i * 2],
                        func=AF.Identity)
                    nc.vector.tensor_copy(
                        out=qt_t[e][:, NB:2 * NB], in_=ps8[i * 2 + 1])

            # x^T exchange: collectives cannot read IO tensors, so mirror
            # the loaded chunks SBUF->DRAM on the Scalar queue. Issued
            # after the Q' sweeps so the 2MB of mirror writes never touch
            # HBM during the input-stream gate; partner x^T is consumed
            # last (B_partner), leaving ~90us of slack.
            for b in range(DT // 2):
                nc.scalar.dma_start(out=x_mir[b * P:(b + 1) * P, :],
                                    in_=xch[b])
            # One merged 2MB gather: the pairwise CC stream is SERIAL
            # (barrier, then each op back-to-back at ~10-17us apiece), so
            # fewer, larger ops shorten the worst-case chain when peer
            # launch skew delays the stream's start barrier.
            nc.gpsimd.collective_compute(
                "AllGather", mybir.AluOpType.bypass,
                replica_groups=GROUPS,
                ins=[x_mir[:, :]],
                outs=[x_recv[:, :]])

        # ---------------- Phases B, C ----------------
        if True:
            phb = res
            # Partner-half import tiles (filled after V'' below).
            xt_par = phb.tile([P, DT // 2, 2 * H], BF16, tag="xp", name="xp")
            v_par = phb.tile([P, HT, D], BF16, tag="vp", name="vp")

            exp_t = [phb.tile([P, Q], BF16, tag=f"exp{j}", name=f"exp{j}")
                     for j in range(ST)]
            y_acc = [phb.tile([P, Q], BF16, tag=f"ya{e}", name=f"ya{e}")
                     for e in range(DT)]

            # f32 per-key-lane partial sums, accumulated on the (idle) DVE;
            # cast to bf16 and reduced across partitions with two cheap
            # bf16 matmuls.
            sumacc = phb.tile([P, Q], F32, tag="sumacc", name="sumacc")
            sumacc_bf = phb.tile([P, Q], BF16, tag="sumbf", name="sumbf")
            ones_b = phb.tile([P, 1], BF16, tag="ones_b", name="ones_b")
            nc.vector.memset(ones_b, 1.0)

            def xkey_sl(e, j):
                # stationary [128 d-rows of e-block, 128 key-tokens]
                if j < HT:
                    return xt_sl(e, slice(j * P, (j + 1) * P))
                jj = j - HT
                base = (e % 2) * H
                return xt_par[:, e // 2, base + jj * P:base + (jj + 1) * P]

            def v_tile(j):
                if j < HT:
                    return v_own[j]
                return v_par[:, j - HT, :]

            # Exchanged data is consumed as late as possible:
            #   B_own -> C_own (own keys only) -> B_partner -> C_partner.
            def b_group(j):
                ps = next_pair(f"bps{j}")
                for e in range(DT):
                    for qb in range(2):
                        nc.tensor.matmul(
                            ps[qb], xkey_sl(e, j),
                            qt_t[e][:, qb * NB:(qb + 1) * NB],
                            start=(e == 0), stop=(e == DT - 1))
                for qb in range(2):
                    sl = slice(qb * NB, (qb + 1) * NB)
                    nc.scalar.activation(
                        out=exp_t[j][:, sl], in_=ps[qb], func=AF.Exp,
                        bias=beta_t[:, j:j + 1])
                    if j == 0:
                        nc.vector.tensor_copy(
                            out=sumacc[:, sl], in_=exp_t[j][:, sl])
                    else:
                        nc.vector.tensor_add(
                            sumacc[:, sl], sumacc[:, sl], exp_t[j][:, sl])

            def c_pass(p, half, last=False):
                # 2 e-tiles x 2 q-halves = 4 PSUM banks, s/av sets
                # alternating so a pass never waits on the previous drain.
                # half 0 writes y_acc; half 1 accumulates into it (DVE) and
                # the finished y^T tiles stream out.
                if p % 2 == 0:
                    ps_o = [scr_ps.tile([P, NB], F32, tag=f"s{i % 2}",
                                        name=f"cps{half}_{p}_{i}")
                            for i in range(4)]
                else:
                    ps_o = [av_ps.tile([P, NB], F32, tag=f"av{i}",
                                       name=f"cps{half}_{p}_{i}")
                            for i in range(4)]
                j0 = half * HT

                def mms(i, qb, j):
                    nc.tensor.matmul(
                        ps_o[i * 2 + qb],
                        v_tile(j)[:, (p * 2 + i) * P:(p * 2 + i + 1) * P],
                        exp_t[j][:, qb * NB:(qb + 1) * NB],
                        start=(j == j0), stop=(j == j0 + HT - 1))

                def drain(i, qb):
                    dp = p * 2 + i
                    sl = slice(qb * NB, (qb + 1) * NB)
                    if half == 0:
                        if qb == 0:
                            nc.scalar.activation(out=y_acc[dp][:, sl],
                                                 in_=ps_o[i * 2],
                                                 func=AF.Identity)
                        else:
                            nc.vector.tensor_copy(out=y_acc[dp][:, sl],
                                                  in_=ps_o[i * 2 + 1])
                    elif last and i == 1 and qb == 1:
                        # Final bank: drain in two 256-col pieces on two DMA
                        # queues so the exposed tail is one quarter-add +
                        # two overlapped small DMAs.
                        for t in range(2):
                            tsl = slice(qb * NB + t * (NB // 2),
                                        qb * NB + (t + 1) * (NB // 2))
                            psl = slice(t * (NB // 2), (t + 1) * (NB // 2))
                            nc.vector.tensor_add(
                                y_acc[dp][:, tsl], y_acc[dp][:, tsl],
                                ps_o[i * 2 + qb][:, psl])
                            eng = nc.scalar if t == 0 else nc.sync
                            eng.dma_start(
                                out=yt[dp * P:(dp + 1) * P, tsl],
                                in_=y_acc[dp][:, tsl])
                    else:
                        nc.vector.tensor_add(
                            y_acc[dp][:, sl], y_acc[dp][:, sl],
                            ps_o[i * 2 + qb])
                        eng = nc.scalar if qb == 0 else nc.sync
                        eng.dma_start(out=yt[dp * P:(dp + 1) * P, sl],
                                      in_=y_acc[dp][:, sl])

                if last:
                    # Bank-major: each bank's 8-key-tile accumulation
                    # finishes 8 matmul slots before the next one, so its
                    # DVE add + y^T DMA overlap the remaining matmuls and
                    # only the final bank's drain is exposed in the tail.
                    for i in range(2):
                        for qb in range(2):
                            for j in range(j0, j0 + HT):
                                mms(i, qb, j)
                            drain(i, qb)
                else:
                    for j in range(j0, j0 + HT):
                        for i in range(2):
                            for qb in range(2):
                                mms(i, qb, j)
                    for i in range(2):
                        for qb in range(2):
                            drain(i, qb)
                return ps_o

            # V''[token, e] -> SBUF (kept) + v_send; AllGather per half.
            # Two d-major 8-bank sweeps (4 token-tiles x 2 e-halves) so the
            # PE consumes wvp[d] in DMA arrival order - sweep 0 starts as
            # soon as wvp chunk 0 lands instead of stalling on the last one.
            for s in range(2):
                ps8 = sweep8(f"vp{s}")
                for d in range(DT):
                    for jj in range(4):
                        j = s * 4 + jj
                        for eb in range(2):
                            nc.tensor.matmul(
                                ps8[jj * 2 + eb],
                                xt_sl(d, slice(j * P, (j + 1) * P)),
                                wvp_sl(d, slice(eb * NB, (eb + 1) * NB)),
                                start=(d == 0), stop=(d == DT - 1))
                for jj in range(4):
                    j = s * 4 + jj
                    nc.scalar.activation(out=v_own[j][:, 0:NB],
                                         in_=ps8[jj * 2], func=AF.Identity)
                    nc.vector.tensor_copy(out=v_own[j][:, NB:2 * NB],
                                          in_=ps8[jj * 2 + 1])
                    nc.sync.dma_start(out=v_send[j * P:(j + 1) * P, :],
                                      in_=v_own[j])
            nc.gpsimd.collective_compute(
                "AllGather", mybir.AluOpType.bypass,
                replica_groups=GROUPS,
                ins=[v_send[:, :]],
                outs=[v_recv[:, :]])

            # Partner-half import: rank parity picks the gathered block.
            pid = nc.sync.partition_id()
            parity = pid % 2
            pbase_x = (1 - parity) * (D // 2)
            nc.sync.dma_start(
                out=xt_par,
                in_=x_recv[bass.ds(pbase_x, D // 2), :].rearrange(
                    "(c p) t -> p c t", p=P))
            pbase_v = (1 - parity) * H
            nc.sync.dma_start(
                out=v_par,
                in_=v_recv[bass.ds(pbase_v, H), :].rearrange(
                    "(j p) e -> p j e", p=P))

            # B_own between the exchanges and C_own: needs no DMA data,
            # so neither a slow input stream nor a late collective can
            # stall the PE here.
            for j in range(HT):
                b_group(j)

            for p in range(4):
                c_pass(p, 0)
            for j in range(HT, ST):
                b_group(j)
            nc.vector.tensor_copy(out=sumacc_bf, in_=sumacc)
            sums_sb = phb.tile([1, Q], F32, tag="sums_sb", name="sums_sb")
            av_saved = None
            for p in range(4):
                ps_o = c_pass(p, 1, last=(p == 3))
                if p == 1:
                    av_saved = ps_o
                if p == 2:
                    # Softmax denominators: two cheap bf16 matmuls into row 0
                    # of pass-1's retired av banks; copies via the ScalarE.
                    for qb in range(2):
                        fs = av_saved[2 + qb][0:1, :]
                        nc.tensor.matmul(
                            fs, ones_b, sumacc_bf[:, qb * NB:(qb + 1) * NB],
                            start=True, stop=True)
                        nc.scalar.activation(
                            out=sums_sb[:, qb * NB:(qb + 1) * NB], in_=fs,
                            func=AF.Identity)
                    nc.sync.dma_start(out=sums[:, :], in_=sums_sb)

    return nc


_NC_CACHE = None


def _get_nc():
    global _NC_CACHE
    if _NC_CACHE is None:
        _NC_CACHE = build_nc()
    return _NC_CACHE


# ---------------------------------------------------------------------------
# Host side
# ---------------------------------------------------------------------------


def _pack_chunks(a):
    """[1024, 1024] (d, cols) -> [512, 2048] 2-d-tile chunk layout."""
    return np.ascontiguousarray(
        a.reshape(4, 2, 128, a.shape[1]).transpose(0, 2, 1, 3)
        .reshape(512, 2 * a.shape[1]))


def _prep_in_maps(x, W_qkv, b_qkv, W_proj, b_proj):
    x = np.asarray(x, dtype=np.float32)
    W_qkv = np.asarray(W_qkv, dtype=np.float32)
    b_qkv = np.asarray(b_qkv, dtype=np.float32)
    W_proj = np.asarray(W_proj, dtype=np.float32)
    b_proj = np.asarray(b_proj, dtype=np.float32)

    scale = 1.0 / math.sqrt(D)
    bf = ml_dtypes.bfloat16
    Wq = W_qkv[:D]
    Wk = W_qkv[D:2 * D]
    Wv = W_qkv[2 * D:]
    b_q = b_qkv[:D]
    b_v = b_qkv[2 * D:]

    M = (Wq.T * scale) @ Wk                    # [d, d']
    wvp = (W_proj @ Wv).T                      # [d, e]
    u = scale * (Wk.T @ b_q)                   # [d]
    b_eff = b_proj + W_proj @ b_v

    m_pack = _pack_chunks(M)
    # split by e'-half so Q' sweep 1's columns stream after sweep 0's
    m0_h = np.ascontiguousarray(np.concatenate(
        [m_pack[:, 0:NB], m_pack[:, D:D + NB]], axis=1)).astype(bf)
    m1_h = np.ascontiguousarray(np.concatenate(
        [m_pack[:, NB:D], m_pack[:, D + NB:2 * D]], axis=1)).astype(bf)
    wvp_h = _pack_chunks(wvp).astype(bf)

    in_maps = []
    for c in range(N_CORES):
        b, h = divmod(c, 2)
        xt_h = _pack_chunks(
            np.ascontiguousarray(x[b, h * H:(h + 1) * H, :].T)).astype(bf)
        beta_all = x[b] @ u                    # [2048] per-key bias
        beta_c = np.concatenate(
            [beta_all[h * H:(h + 1) * H],
             beta_all[(1 - h) * H:(2 - h) * H]]).reshape(ST, P).T
        in_maps.append({"xt": xt_h, "m0": m0_h, "m1": m1_h, "wvp": wvp_h,
                        "beta": np.ascontiguousarray(beta_c,
                                                     dtype=np.float32)})
    return in_maps, b_eff


def _postprocess(results, b_eff):
    y = np.empty((4, S, D), dtype=np.float32)
    for c in range(N_CORES):
        b, h = divmod(c, 2)
        ytc = results[c]["yt"].astype(np.float32)  # [D(e), Q] unnormalized
        sc = results[c]["sums"][0]                 # [Q] softmax denominators
        y[b, h * Q:(h + 1) * Q, :] = ytc.T / sc[:, None] + b_eff[None, :]
    return y


def kernel(x, W_qkv, b_qkv, W_proj, b_proj, **run_kwargs):
    nc = _get_nc()
    in_maps, b_eff = _prep_in_maps(x, W_qkv, b_qkv, W_proj, b_proj)
    last_exc = None
    for attempt in range(3):
        try:
            res = run_bass_kernel_spmd(nc, in_maps,
                                       core_ids=list(range(N_CORES)),
                                       **run_kwargs)
            break
        except Exception as exc:  # transient NRT device errors
            last_exc = exc
            import time
            time.sleep(2.0 * (attempt + 1))
    else:
        raise last_exc
    y = _postprocess(res.results, b_eff)
    kernel.last_result = res
    return y
